# revision 21
# baseline (speedup 1.0000x reference)
"""Trainium2 Bass kernel for nn_AttachmentPredictor (mask-packed).

Only rows with mask=True contribute to the output (masked exp-norm over
head positions), so the host gathers just those rows (~50%) per core,
batch-major, padded to nblk*512 rows, and the device runs the dense
pipeline on the packed rows:

  stage1: psum[jt] += Wh[dk,jt] @ xT[dk, :]  (feature-major, 512 rows/blk)
          += bias_bm[:, jt]^T @ E            (per-batch bias via one-hot E)
  tanh -> c1; stages 2/3: hidden layers, tanh -> c2, c3
  scorer: [1,512] psum rows of scores via M=1 matmuls -> DMA raw scores.

Host: exp(scores), scatter to (b, s), per-batch normalize. The NEFF is
mask-agnostic (E carries the row->batch map); only nblk (padded block
count) specializes the build.

Matmuls run as float32r (TF32-like, full PE rate) by default; stage 1
optionally fp8e4m3 with DoubleRow (2 k-tiles per MM) per OPTS.
"""

import ml_dtypes
import numpy as np

import concourse.bass as bass
import concourse.mybir as mybir
import concourse.tile as tile
from concourse import bass_utils
from concourse.bass import ts

F32 = mybir.dt.float32
F32R = mybir.dt.float32r
BF16 = mybir.dt.bfloat16
FP8 = mybir.dt.float8e4
AF = mybir.ActivationFunctionType
DR = mybir.MatmulPerfMode.DoubleRow

B, S, D, P = 256, 256, 1024, 512
NCORES = 8
BC = B // NCORES            # 32 batches per core
KD = D // 128               # 8 k-tiles over D
KP = P // 128               # 4 k-tiles over P
EPS = 1e-7

OPTS = {
    "s1_dtype": "fp8",    # stage-1 x/Wh: "f32r" | "bf16" | "fp8" (=> DoubleRow)
    "mm_dtype": "bf16",   # stages 2/3, scorer, bias path, E: "f32r" | "bf16"
    "group": 8,           # blocks per group; stages interleave across the
                          # group so ACT latency hides under sibling matmuls
    "xr_bufs": 9,
    "c_bufs": 33,
    "ps_bufs": 8,
}

_DT = {"f32r": F32R, "bf16": BF16, "f32": F32, "fp8": FP8}
_NPDT = {"f32r": np.float32, "bf16": ml_dtypes.bfloat16, "f32": np.float32,
         "fp8": ml_dtypes.float8_e4m3}


# ---------------------------------------------------------------------------
# walrus in this container accepts at most ONE sync wait per instruction;
# split extra waits onto preceding NoOps on the same engine.
def _split_waits(nc, maxw=1):
    ctr = 0
    for f in nc.m.functions:
        for blk in f.blocks:
            insts = blk.instructions
            newlist = []
            changed = False
            for inst in insts:
                si = inst.sync_info
                if si is not None and len(si.on_wait) > maxw:
                    waits = list(si.on_wait)
                    keep = waits[len(waits) - maxw:]
                    extra = waits[: len(waits) - maxw]
                    for j in range(0, len(extra), maxw):
                        ctr += 1
                        newlist.append(
                            mybir.InstNoOp(
                                name=f"waitsplit-{ctr}",
                                engine=inst.engine,
                                ins=[],
                                outs=[],
                                sync_info=mybir.SyncInfo(
                                    on_wait=extra[j: j + maxw], on_update=[]
                                ),
                            )
                        )
                    inst.sync_info = mybir.SyncInfo(
                        on_wait=keep, on_update=list(si.on_update)
                    )
                    changed = True
                newlist.append(inst)
            if changed:
                insts[:] = newlist


# ---------------------------------------------------------------------------
def _build(opts=None, nblk=9, reps=1, split=True):
    opts = dict(OPTS, **(opts or {}))
    nc = bass.Bass("TRN2", target_bir_lowering=False, debug=False)

    S1DT = _DT[opts["s1_dtype"]]
    MMDT = _DT[opts["mm_dtype"]]
    fp8_s1 = opts["s1_dtype"] == "fp8"

    xT_d = nc.dram_tensor("xT", [nblk, 128, KD * 512], S1DT,
                          kind="ExternalInput").ap()
    e_d = nc.dram_tensor("esel", [nblk, BC, 512], MMDT,
                         kind="ExternalInput").ap()
    xp_d = nc.dram_tensor("xprep", [D, BC], MMDT, kind="ExternalInput").ap()
    xc_d = nc.dram_tensor("xchild", [D, BC], MMDT, kind="ExternalInput").ap()
    wh_d = nc.dram_tensor("wh", [D, P], S1DT, kind="ExternalInput").ap()
    wp_d = nc.dram_tensor("wp", [D, P], MMDT, kind="ExternalInput").ap()
    wc_d = nc.dram_tensor("wc", [D, P], MMDT, kind="ExternalInput").ap()
    w0_d = nc.dram_tensor("w0", [P, P], MMDT, kind="ExternalInput").ap()
    w1_d = nc.dram_tensor("w1", [P, P], MMDT, kind="ExternalInput").ap()
    sc_d = nc.dram_tensor("scT", [128, KP], MMDT, kind="ExternalInput").ap()
    out_d = nc.dram_tensor("scores", [nblk, 512], F32,
                           kind="ExternalOutput").ap()

    with tile.TileContext(nc) as tc:
        with (
            tc.tile_pool(name="consts", bufs=1) as consts,
            tc.tile_pool(name="ssb", bufs=3) as spool,
            tc.tile_pool(name="xr", bufs=opts["xr_bufs"]) as xpool,
            tc.tile_pool(name="acts", bufs=opts["c_bufs"]) as cpool,
            tc.tile_pool(name="ps", bufs=opts["ps_bufs"], space="PSUM") as pspool,
        ):
            # ---- constants -------------------------------------------------
            def load_packed(dram, k, n, dt, tag):
                t = consts.tile([128, k * n], dt, tag=tag)
                nc.sync.dma_start(
                    t[:].rearrange("p (k n) -> p k n", n=n),
                    dram.rearrange("(k p) n -> p k n", p=128),
                )
                return t

            wh_r = load_packed(wh_d, KD, P, S1DT, "wh")      # [128, KD*512]
            xp_r = load_packed(xp_d, KD, BC, MMDT, "xp")     # [128, KD*32]
            xc_r = load_packed(xc_d, KD, BC, MMDT, "xc")
            w0_r = load_packed(w0_d, KP, P, MMDT, "w0")
            w1_r = load_packed(w1_d, KP, P, MMDT, "w1")
            sc_r = consts.tile([128, KP], MMDT, tag="sc")
            nc.sync.dma_start(sc_r[:], sc_d[:])

            # ---- per-batch bias, batch-major [32, 512] ---------------------
            psb = pspool.tile([BC, P], F32, tag="ps", name="psb")
            for i, (xs, w_d) in enumerate(((xp_r, wp_d), (xc_r, wc_d))):
                for dk in range(KD):
                    wst = spool.tile([128, P], MMDT, tag="wst")
                    nc.sync.dma_start(wst[:], w_d[dk * 128: (dk + 1) * 128, :])
                    nc.tensor.matmul(
                        psb[:],
                        xs[:, dk * BC: (dk + 1) * BC],
                        wst[:],
                        start=(i == 0 and dk == 0),
                        stop=(i == 1 and dk == KD - 1),
                    )
            bias_sb = consts.tile([BC, P], MMDT, tag="bias")
            nc.vector.tensor_copy(bias_sb[:], psb[:])

            # ---- main loop over groups of packed blocks (512 rows each) ----
            G = opts["group"]
            for _rep in range(reps):
                groups = [list(range(g, min(g + G, nblk)))
                          for g in range(0, nblk, G)]
                for grp in groups:
                    xrs, esbs = {}, {}
                    for blk in grp:
                        xr = xpool.tile([128, KD * 512], S1DT, tag="xr")
                        hw = KD * 512 // 2
                        for h in range(2):
                            nc.sync.dma_start(
                                xr[:, h * hw: (h + 1) * hw],
                                xT_d[blk, :, h * hw: (h + 1) * hw],
                            )
                        xrs[blk] = xr
                        e_sb = spool.tile([BC, 512], MMDT, tag="esb",
                                          bufs=G + 1, name=f"esb_{blk}")
                        nc.sync.dma_start(e_sb[:], e_d[blk])
                        esbs[blk] = e_sb

                    # stage 1 (+ bias via one-hot E), tanh -> c1.
                    # Pairs of blocks, jt-major within the pair: consecutive
                    # matmuls share the stationary Wh k-pair, so LDWEIGHTS is
                    # elided on the second (DoubleRow LDW otherwise exposes
                    # ~200ns: it fills both weight buffers, defeating the
                    # background-load overlap). E-matmuls batch per block (2
                    # DR<->normal mode switches); ACTs of block A drain under
                    # block B's E batch, so the next pair starts stall-free.
                    cs = {blk: [] for blk in grp}
                    whv = wh_r[:].rearrange("p (k n) -> p k n", n=P)
                    pairs = [grp[i: i + 2] for i in range(0, len(grp), 2)]
                    for pair in pairs:
                        ps1m = {
                            blk: [pspool.tile([128, 512], F32, tag="ps",
                                              name=f"ps1_{blk}_{jt}")
                                  for jt in range(KP)]
                            for blk in pair
                        }
                        for jt in range(KP):
                            if fp8_s1:
                                for a in range(KD // 2):
                                    for bi, blk in enumerate(pair):
                                        xrv = xrs[blk][:].rearrange(
                                            "p (k n) -> p k n", n=512)
                                        inst = nc.tensor.matmul(
                                            ps1m[blk][jt][:],
                                            whv[:, 2 * a: 2 * a + 2,
                                                jt * 128: (jt + 1) * 128],
                                            xrv[:, 2 * a: 2 * a + 2, :],
                                            start=(a == 0),
                                            stop=False,
                                            perf_mode=DR,
                                        )
                                        if bi > 0:
                                            inst.ldweights = False
                            else:
                                for dk in range(KD):
                                    for bi, blk in enumerate(pair):
                                        inst = nc.tensor.matmul(
                                            ps1m[blk][jt][:],
                                            wh_r[:, dk * P + jt * 128:
                                                 dk * P + (jt + 1) * 128],
                                            xrs[blk][:, dk * 512:
                                                      (dk + 1) * 512],
                                            start=(dk == 0),
                                            stop=False,
                                        )
                                        if bi > 0:
                                            inst.ldweights = False
                        for blk in pair:
                            for jt in range(KP):
                                nc.tensor.matmul(
                                    ps1m[blk][jt][:],
                                    bias_sb[:, jt * 128: (jt + 1) * 128],
                                    esbs[blk][:],
                                    start=False,
                                    stop=True,
                                )
                        for blk in pair:
                            for jt in range(KP):
                                ct = cpool.tile([128, 512], MMDT, tag="c1")
                                nc.scalar.activation(ct[:],
                                                     ps1m[blk][jt][:],
                                                     AF.Tanh)
                                cs[blk].append(ct)

                    # stages 2, 3 (block-major within the group)
                    for stage, w_r in ((2, w0_r), (3, w1_r)):
                        for blk in grp:
                            c_out = []
                            for qt in range(KP):
                                ps2 = pspool.tile([128, 512], F32, tag="ps",
                                                  name=f"ps{stage}_{blk}_{qt}")
                                for jk in range(KP):
                                    nc.tensor.matmul(
                                        ps2[:],
                                        w_r[:, jk * P + qt * 128:
                                            jk * P + (qt + 1) * 128],
                                        cs[blk][jk][:],
                                        start=(jk == 0),
                                        stop=(jk == KP - 1),
                                    )
                                ct = cpool.tile([128, 512], MMDT,
                                                tag=f"c{stage}")
                                nc.scalar.activation(ct[:], ps2[:], AF.Tanh)
                                c_out.append(ct)
                            cs[blk] = c_out

                    # scorer -> raw scores rows
                    for blk in grp:
                        pss = pspool.tile([1, 512], F32, tag="ps",
                                          name=f"pss_{blk}")
                        for qk in range(KP):
                            nc.tensor.matmul(
                                pss[:],
                                sc_r[:, qk: qk + 1],
                                cs[blk][qk][:],
                                start=(qk == 0),
                                stop=(qk == KP - 1),
                            )
                        so = spool.tile([1, 512], F32, tag="so",
                                        bufs=G + 1, name=f"so_{blk}")
                        nc.vector.tensor_copy(so[:], pss[:])
                        nc.sync.dma_start(out_d[blk: blk + 1, :], so[:])

    if split:
        _split_waits(nc)
    return nc


# ---------------------------------------------------------------------------
def _host_prep(x, proj_head, proj_prep, proj_child, hidden_layers, scorer, mask,
               opts=None):
    opts = dict(OPTS, **(opts or {}))
    s1_np = _NPDT[opts["s1_dtype"]]
    mm_np = _NPDT[opts["mm_dtype"]]
    x = np.asarray(x, np.float32)
    mask = np.asarray(mask)
    wh = np.ascontiguousarray(np.asarray(proj_head, np.float32).astype(s1_np))
    wp = np.ascontiguousarray(np.asarray(proj_prep, np.float32).astype(mm_np))
    wc = np.ascontiguousarray(np.asarray(proj_child, np.float32).astype(mm_np))
    hl = np.asarray(hidden_layers, np.float32)
    w0 = np.ascontiguousarray(hl[0].astype(mm_np))
    w1 = np.ascontiguousarray(hl[1].astype(mm_np))
    scT = np.ascontiguousarray(
        np.asarray(scorer, np.float32).reshape(KP, 128).T.astype(mm_np)
    )  # [128, 4]

    # Balance row counts across cores: LPT-assign batches (32 per core) so
    # the max per-core packed row count is minimal (usually fits 8 blocks
    # instead of 9 for a ~50% mask). The NEFF is unchanged; the batch
    # permutation is undone in the output scatter via metas.
    counts = mask[:, : S - 2].sum(axis=1)
    order = np.argsort(-counts, kind="stable")
    core_sum = np.zeros(NCORES, np.int64)
    core_n = np.zeros(NCORES, np.int64)
    assign = [[] for _ in range(NCORES)]
    for b in order:
        open_cores = np.nonzero(core_n < BC)[0]
        c = open_cores[np.argmin(core_sum[open_cores])]
        assign[c].append(b)
        core_sum[c] += counts[b]
        core_n[c] += 1
    batches = [np.asarray(a) for a in assign]   # global batch ids per core

    metas = []
    for c in range(NCORES):
        mb = mask[batches[c]][:, : S - 2]
        b_idx, s_idx = np.nonzero(mb)          # batch-major order (local)
        metas.append((batches[c], b_idx, s_idx, len(b_idx)))
    nblk = max(1, max((m[3] + 511) // 512 for m in metas))

    in_maps = []
    for c in range(NCORES):
        gbatch, b_idx, s_idx, nrows = metas[c]
        xb = x[gbatch]                                      # [32, 256, 1024]
        xP = np.zeros((nblk * 512, D), np.float32)
        xP[:nrows] = xb[b_idx, s_idx]
        if s1_np is ml_dtypes.float8_e4m3:
            np.clip(xP, -240.0, 240.0, out=xP)
        xTc = np.ascontiguousarray(
            xP.reshape(nblk, 512, KD, 128).transpose(0, 3, 2, 1).astype(s1_np)
        ).reshape(nblk, 128, KD * 512)
        esel = np.zeros((nblk, BC, 512), mm_np)
        rr = np.arange(nrows)
        esel[rr // 512, b_idx, rr % 512] = 1
        xpc = np.ascontiguousarray(xb[:, S - 2, :].T.astype(mm_np))  # [1024, 32]
        xcc = np.ascontiguousarray(xb[:, S - 1, :].T.astype(mm_np))
        in_maps.append(
            {
                "xT": xTc, "esel": esel, "xprep": xpc, "xchild": xcc,
                "wh": wh, "wp": wp, "wc": wc, "w0": w0, "w1": w1, "scT": scT,
            }
        )
    return in_maps, metas, nblk


_NC_CACHE = {}


def _get_nc(opts=None, nblk=9):
    opts = dict(OPTS, **(opts or {}))
    key = (opts["s1_dtype"], opts["mm_dtype"], opts["group"], nblk)
    if key not in _NC_CACHE:
        _NC_CACHE[key] = _build(opts, nblk=nblk)
    return _NC_CACHE[key]


def kernel(x, proj_head, proj_prep, proj_child, hidden_layers, scorer, mask):
    in_maps, metas, nblk = _host_prep(
        x, proj_head, proj_prep, proj_child, hidden_layers, scorer, mask
    )
    nc = _get_nc(nblk=nblk)
    res = bass_utils.run_bass_kernel_spmd(
        nc, in_maps, core_ids=list(range(NCORES))
    )
    out = np.zeros((B, S - 2), np.float32)
    for c in range(NCORES):
        gbatch, b_idx, s_idx, nrows = metas[c]
        sc = res.results[c]["scores"].reshape(-1)[:nrows].astype(np.float64)
        me = np.zeros((BC, S - 2))
        me[b_idx, s_idx] = np.exp(sc)
        sums = me.sum(axis=1, keepdims=True) + EPS
        out[gbatch] = (me / sums).astype(np.float32)
    return out


if __name__ == "__main__":
    rng = np.random.default_rng(0)
    x = rng.standard_normal((B, S, D)).astype(np.float32)
    u = lambda shp: rng.uniform(-0.05, 0.05, shp).astype(np.float32)
    inputs = dict(
        x=x, proj_head=u((D, P)), proj_prep=u((D, P)), proj_child=u((D, P)),
        hidden_layers=u((2, P, P)), scorer=u((P,)),
        mask=rng.integers(0, 2, (B, S)).astype(bool),
    )
    out = kernel(**inputs)
    print("kernel out", out.shape, out.dtype, out[:2, :4])
